# revision 24
# baseline (speedup 1.0000x reference)
"""Trainium2 Bass kernel for nn_LogicLayer (difflogic soft-logic layer).

Math: out[i, j] = c0[j] + ca[j]*a + cb[j]*b + cab[j]*a*b
  where a = x[i, idx_a[j]], b = x[i, idx_b[j]] and the c* coefficients are
  linear combinations of softmax(weights[j]) (all 16 soft logic gates are
  polynomials over the basis {1, a, b, a*b}).

Sharding: output neurons split across 8 NeuronCores (2048 each), transposed
layout (neuron on partition axis, batch on free axis).

Byte-shrunk data path (v2): x in [0,1) is quantized on the host to int8
(q = round(254*x) - 127, so a = q/254 + 0.5 exactly), halving the a-side
gather to 2KB/row.  The b-side stays fp16.  The output is stored as int8 in
a per-neuron quantized space oq = (out - mn)/step - 120 whose affine map is
folded into the polynomial coefficients on device (the corner values of the
bilinear map bound out exactly, so |oq| <= 120 by construction).  The host
dequantizes with the same per-neuron step/mn.  Total per-core HBM traffic:
4MB (a int8) + 8MB (b fp16) + 4MB (out int8) = 16MB vs 24MB for the fp16
path.

Engine budget per [128, 2048] tile (cost-model ns):
  ScalarE : p = C2*qa + B2            (act, int8-in, fp16-out)  1707
            r = A2*qa + c02 on RSPLIT tiles (2nd act)           1707
  GpSimd  : r on the other tiles (tensor_scalar, int8-in)       2845
            + 8 dma_gather issues/iter
  DVE     : w = p * b16   (tt 2x)                               1067
            o = w + r -> int8 (tt, 1x due to int8 out)          2133
DVE ~51us is the projected wall; DMA 16MB ~48us.
"""
import numpy as np

import concourse.bacc as bacc
import concourse.mybir as mybir
import concourse.tile as tile
from concourse import bass_utils

F32 = mybir.dt.float32
F16 = mybir.dt.float16
I16 = mybir.dt.int16
I8 = mybir.dt.int8

B = 2048
IN_DIM = 8192
OUT_DIM = 16384
NCORES = 8
OUTC = OUT_DIM // NCORES     # neurons per core
NT = OUTC // 128             # 128-neuron tiles per core
IDX_PER = 512                # indices per dma_gather call (4 tiles)
NQ = 4                       # SWDGE queues

QS = 254.0                   # a = q/QS + 0.5, q in [-127, 127]
OQ_RANGE = 240.0             # oq spans [-120, 120]
OQ_HALF = 120.0
EPS = 1e-6

_CACHE = {}
SORT_B = True   # host-side neuron sort by idx_b for HBM gather locality


N_I8 = 6                     # tiles (of NT=16) stored as int8 (rest fp16)
N_RDVE = 6                   # tiles whose r-op runs on DVE (rest ScalarE)


def _build_nc(repeats=1, out_i8=True, a_i8=True, nq=NQ, idx_per=IDX_PER,
              rsplit=None, gbufs=4, mode="full", n_i8=N_I8, n_rdve=N_RDVE,
              cbufs=4, sp=0, n_ogps=0):
    """n_i8: tiles NT-n_i8..NT-1 store int8 (quantized); rest store fp16.
    n_rdve: r-op on DVE for even tiles 0,2,..; rest on ScalarE.
    rsplit (legacy probes): if set, overrides r placement:
      <0 all-DVE, k in [0,16] -> tiles t%16<k on ScalarE, rest GpSimd.
    mode: 'full' | 'compute' (no gathers; memset inputs once) |
    'dma' (gathers + stores of a memset tile; no compute)."""
    if not out_i8:
        n_i8 = 0
    nc = bacc.Bacc("TRN2", target_bir_lowering=False, debug=False,
                   enable_asserts=False, num_swdge_queues=nq)
    if a_i8:
        xq_d = nc.dram_tensor("xq", [IN_DIM, B], I8, kind="ExternalInput")
    else:
        xq_d = None
    xT_d = nc.dram_tensor("xT", [IN_DIM, B], F16, kind="ExternalInput")
    w_d = nc.dram_tensor("wre", [128, NT * 16], F32, kind="ExternalInput")
    idxa_d = nc.dram_tensor("idxa", [128, NT * 8], I16, kind="ExternalInput")
    idxb_d = nc.dram_tensor("idxb", [128, NT * 8], I16, kind="ExternalInput")
    n_f16 = NT - n_i8
    out_d = out8_d = None
    if n_f16:
        out_d = nc.dram_tensor("outT", [n_f16 * 128, B], F16,
                               kind="ExternalOutput")
    if n_i8:
        out8_d = nc.dram_tensor("outT8", [n_i8 * 128, B], I8,
                                kind="ExternalOutput")

    add = mybir.AluOpType.add
    sub = mybir.AluOpType.subtract
    mult = mybir.AluOpType.mult
    amax = mybir.AluOpType.max
    amin = mybir.AluOpType.min
    ident = mybir.ActivationFunctionType.Identity

    with tile.TileContext(nc) as tc:
        with (
            tc.tile_pool(name="const", bufs=1) as cpool,
            tc.tile_pool(name="gata", bufs=gbufs) as gapool,
            tc.tile_pool(name="gatb", bufs=gbufs) as gbpool,
            tc.tile_pool(name="pr", bufs=cbufs) as prpool,
            tc.tile_pool(name="out", bufs=cbufs) as opool,
        ):
            idxa_sb = cpool.tile([128, NT * 8], I16, tag="idxa")
            nc.sync.dma_start(idxa_sb[:], idxa_d.ap())
            idxb_sb = cpool.tile([128, NT * 8], I16, tag="idxb")
            nc.sync.dma_start(idxb_sb[:], idxb_d.ap())
            w_sb = cpool.tile([128, NT * 16], F32, tag="w")
            nc.sync.dma_start(w_sb[:], w_d.ap())

            # ---- per-neuron coefficients from softmax(weights) ----
            # weights ~ N(0,1): exp() cannot overflow f32, skip max-subtraction
            e = cpool.tile([128, NT * 16], F32, tag="e")
            nc.scalar.activation(e[:], w_sb[:], mybir.ActivationFunctionType.Exp)
            e3 = e[:].rearrange("p (t g) -> p t g", g=16)

            s = cpool.tile([128, NT], F32, tag="s")
            nc.vector.tensor_reduce(s[:], e3, mybir.AxisListType.X, add)
            r_ = cpool.tile([128, NT], F32, tag="r")
            nc.vector.reciprocal(r_[:], s[:])

            def E(g):
                return e3[:, :, g]

            def tt(out, a_, b_, op):
                nc.vector.tensor_tensor(out, a_, b_, op)

            def ts(out, in_, s1, s2, op0, op1):
                nc.vector.tensor_scalar(out, in_, s1, s2, op0, op1)

            def ctile(tag):
                return cpool.tile([128, NT], F32, tag=tag, name=tag)

            s67 = ctile("s67"); tt(s67[:], E(6), E(7), add)
            s89 = ctile("s89"); tt(s89[:], E(8), E(9), add)
            s1011 = ctile("s1011"); tt(s1011[:], E(10), E(11), add)
            s1213 = ctile("s1213"); tt(s1213[:], E(12), E(13), add)

            c0 = ctile("c0")
            tt(c0[:], s89[:], s1011[:], add)
            tt(c0[:], c0[:], s1213[:], add)
            tt(c0[:], c0[:], E(14), add)
            tt(c0[:], c0[:], E(15), add)

            ca = ctile("ca")
            tt(ca[:], E(2), E(3), add)
            tt(ca[:], ca[:], s67[:], add)
            tt(ca[:], ca[:], s89[:], sub)
            tt(ca[:], ca[:], s1213[:], sub)

            cb = ctile("cb")
            tt(cb[:], E(4), E(5), add)
            tt(cb[:], cb[:], s67[:], add)
            tt(cb[:], cb[:], s89[:], sub)
            tt(cb[:], cb[:], s1011[:], sub)

            cab = ctile("cab")
            tt(cab[:], E(1), E(2), sub)
            tt(cab[:], cab[:], E(4), sub)
            tt(cab[:], cab[:], s67[:], sub)
            tt(cab[:], cab[:], E(6), sub)
            tt(cab[:], cab[:], s89[:], add)
            tt(cab[:], cab[:], E(9), add)
            tt(cab[:], cab[:], E(11), add)
            tt(cab[:], cab[:], E(13), add)
            tt(cab[:], cab[:], E(14), sub)

            for t_ in (c0, ca, cb, cab):
                tt(t_[:], t_[:], r_[:], mult)

            # ---- fold a-dequant (a = qa/QS + 0.5) ----
            # out = c01 + A1*qa + B1*b + C1*qa*b
            if a_i8:
                A1 = ctile("A1"); ts(A1[:], ca[:], 1.0 / QS, 0.0, mult, add)
                C1 = ctile("C1"); ts(C1[:], cab[:], 1.0 / QS, 0.0, mult, add)
                B1 = ctile("B1"); ts(B1[:], cab[:], 0.5, 0.0, mult, add)
                tt(B1[:], B1[:], cb[:], add)
                c01 = ctile("c01"); ts(c01[:], ca[:], 0.5, 0.0, mult, add)
                tt(c01[:], c01[:], c0[:], add)
            else:
                A1, B1, C1, c01 = ca, cb, cab, c0

            if out_i8:
                # ---- per-neuron output quantization from corner bounds ----
                # corners of the bilinear map over (a,b) in [0,1]^2
                k2 = ctile("k2"); tt(k2[:], c0[:], ca[:], add)
                k3 = ctile("k3"); tt(k3[:], c0[:], cb[:], add)
                k4 = ctile("k4"); tt(k4[:], k2[:], cb[:], add)
                tt(k4[:], k4[:], cab[:], add)
                mn = ctile("mn")
                tt(mn[:], c0[:], k2[:], amin)
                tt(mn[:], mn[:], k3[:], amin)
                tt(mn[:], mn[:], k4[:], amin)
                mx = ctile("mx")
                tt(mx[:], c0[:], k2[:], amax)
                tt(mx[:], mx[:], k3[:], amax)
                tt(mx[:], mx[:], k4[:], amax)
                rng = ctile("rng"); tt(rng[:], mx[:], mn[:], sub)
                ts(rng[:], rng[:], 1.0, EPS, mult, add)
                inv = ctile("inv")
                nc.vector.reciprocal(inv[:], rng[:])
                ts(inv[:], inv[:], OQ_RANGE, 0.0, mult, add)
                # oq = (out - mn)*inv - OQ_HALF
                A2 = ctile("A2"); tt(A2[:], A1[:], inv[:], mult)
                B2 = ctile("B2"); tt(B2[:], B1[:], inv[:], mult)
                C2 = ctile("C2"); tt(C2[:], C1[:], inv[:], mult)
                c02 = ctile("c02"); tt(c02[:], c01[:], mn[:], sub)
                tt(c02[:], c02[:], inv[:], mult)
                ts(c02[:], c02[:], 1.0, -OQ_HALF, mult, add)
            else:
                A2, B2, C2, c02 = A1, B1, C1, c01

            # ---- gather + compute + store ----
            tpc = idx_per // 128     # tiles per gather call
            ncalls = NT // tpc
            a_dt = I8 if a_i8 else F16
            ga0 = gb0 = o0 = None
            if mode == "compute":
                ga0 = cpool.tile([128, tpc, B], a_dt, tag="ga0")
                nc.vector.memset(ga0[:], 3)
                gb0 = cpool.tile([128, tpc, B], F16, tag="gb0")
                nc.vector.memset(gb0[:], 0.5)
            o0 = o08 = None
            if mode == "dma":
                if n_f16:
                    o0 = cpool.tile([128, B], F16, tag="o0")
                    nc.vector.memset(o0[:], 1)
                if n_i8:
                    o08 = cpool.tile([128, B], I8, tag="o08")
                    nc.vector.memset(o08[:], 1)
            for rep in range(repeats):
                gas, gbs = [], []
                for call in range(ncalls):
                    isl = slice(call * idx_per // 16, (call + 1) * idx_per // 16)
                    if mode == "compute":
                        gas.append(ga0)
                        gbs.append(gb0)
                        continue
                    ga = gapool.tile([128, tpc, B], a_dt, tag="ga")
                    nc.gpsimd.dma_gather(
                        ga[:], (xq_d if a_i8 else xT_d).ap(), idxa_sb[:, isl],
                        idx_per, idx_per, B,
                        queue_num=(2 * call) % nq, single_packet=bool(sp))
                    gb = gbpool.tile([128, tpc, B], F16, tag="gb")
                    nc.gpsimd.dma_gather(
                        gb[:], xT_d.ap(), idxb_sb[:, isl],
                        idx_per, idx_per, B,
                        queue_num=(2 * call + 1) % nq, single_packet=bool(sp))
                    gas.append(ga)
                    gbs.append(gb)
                rdve = set(range(0, 2 * n_rdve, 2)) if rsplit is None else set()
                for call in range(ncalls):
                    ga, gb = gas[call], gbs[call]
                    for k in range(tpc):
                        t = call * tpc + k
                        is8 = t >= n_f16
                        if mode == "dma":
                            if is8:
                                nc.sync.dma_start(
                                    out8_d.ap()[(t - n_f16) * 128:
                                                (t - n_f16 + 1) * 128, :],
                                    o08[:])
                            else:
                                nc.sync.dma_start(
                                    out_d.ap()[t * 128:(t + 1) * 128, :],
                                    o0[:])
                            continue
                        qa_ap = ga[:, k, :]
                        b_ap = gb[:, k, :]
                        cA, cB, cC, cc0 = ((A2, B2, C2, c02) if is8
                                           else (A1, B1, C1, c01))
                        # p = C*qa + B   (ScalarE act, int8 input)
                        p = prpool.tile([128, B], F16, tag="p")
                        nc.scalar.activation(p[:], qa_ap, ident,
                                             bias=cB[:, t:t + 1],
                                             scale=cC[:, t:t + 1])
                        # r = A*qa + c0
                        r2 = prpool.tile([128, B], F16, tag="r2")
                        if (rsplit is None and t in rdve) or \
                                (rsplit is not None and rsplit < 0):
                            nc.vector.tensor_scalar(r2[:], qa_ap,
                                                    cA[:, t:t + 1],
                                                    cc0[:, t:t + 1],
                                                    mult, add)
                        elif rsplit is None or (t % 16) < rsplit:
                            nc.scalar.activation(r2[:], qa_ap, ident,
                                                 bias=cc0[:, t:t + 1],
                                                 scale=cA[:, t:t + 1])
                        else:
                            nc.gpsimd.tensor_scalar(r2[:], qa_ap,
                                                    cA[:, t:t + 1],
                                                    cc0[:, t:t + 1],
                                                    mult, add)
                        # w = p * b16  (DVE 2x)
                        w_ = opool.tile([128, B], F16, tag="wk")
                        tt(w_[:], p[:], b_ap, mult)
                        # o = w + r  (DVE; int8 out on the n_i8 tail tiles;
                        # first n_ogps f16 tiles ride GpSimd instead)
                        o = opool.tile([128, B], I8 if is8 else F16,
                                       tag="o8" if is8 else "o16")
                        if not is8 and t < n_ogps:
                            nc.gpsimd.tensor_tensor(o[:], w_[:], r2[:], add)
                        else:
                            tt(o[:], w_[:], r2[:], add)
                        if is8:
                            nc.sync.dma_start(
                                out8_d.ap()[(t - n_f16) * 128:
                                            (t - n_f16 + 1) * 128, :], o[:])
                        else:
                            nc.sync.dma_start(
                                out_d.ap()[t * 128:(t + 1) * 128, :], o[:])

    nc.compile()
    return nc


def _wrap_idxs(idx):
    """[n] -> [128, n//16] int16: wrapped[p, s] = idx[s*16 + p%16]."""
    n = idx.shape[0]
    w16 = idx.reshape(n // 16, 16).T.astype(np.int16)
    return np.tile(w16, (8, 1))


def _coef_from_weights(wc):
    """Host-side softmax coefficients for OUTC neurons: c0, ca, cb, cab."""
    wc = wc.astype(np.float64)
    e = np.exp(wc - wc.max(axis=1, keepdims=True))
    E = e / e.sum(axis=1, keepdims=True)
    c0 = E[:, 8:16].sum(1)
    ca = E[:, 2] + E[:, 3] + E[:, 6] + E[:, 7] - E[:, 8] - E[:, 9] - E[:, 12] - E[:, 13]
    cb = E[:, 4] + E[:, 5] + E[:, 6] + E[:, 7] - E[:, 8] - E[:, 9] - E[:, 10] - E[:, 11]
    cab = (E[:, 1] - E[:, 2] - E[:, 4] - 2 * E[:, 6] - E[:, 7] + E[:, 8]
           + 2 * E[:, 9] + E[:, 11] + E[:, 13] - E[:, 14])
    return c0, ca, cb, cab


def _host_prep(x, weights, idx_a, idx_b):
    x = np.asarray(x, dtype=np.float32)
    xT16 = np.ascontiguousarray(x.T.astype(np.float16))
    xq = np.ascontiguousarray(
        (np.rint(x.T * QS) - 127.0).astype(np.int8))
    weights = np.asarray(weights, dtype=np.float32)
    idx_a = np.asarray(idx_a)
    idx_b = np.asarray(idx_b)
    in_maps = []
    deq = []
    perms = []
    for c in range(NCORES):
        lo = c * OUTC
        ia = idx_a[lo:lo + OUTC]
        ib = idx_b[lo:lo + OUTC]
        wsl = weights[lo:lo + OUTC]
        if SORT_B:
            perm = np.argsort(ib, kind="stable")
            ia = ia[perm]
            ib = ib[perm]
            wsl = wsl[perm]
        else:
            perm = np.arange(OUTC)
        perms.append(perm)
        idxa_w = np.ascontiguousarray(np.concatenate(
            [_wrap_idxs(ia[s * IDX_PER:(s + 1) * IDX_PER])
             for s in range(OUTC // IDX_PER)], axis=1))
        idxb_w = np.ascontiguousarray(np.concatenate(
            [_wrap_idxs(ib[s * IDX_PER:(s + 1) * IDX_PER])
             for s in range(OUTC // IDX_PER)], axis=1))
        wre = np.ascontiguousarray(
            wsl.reshape(NT, 128, 16).transpose(1, 0, 2).reshape(128, NT * 16))
        in_maps.append({"xq": xq, "xT": xT16, "wre": wre,
                        "idxa": idxa_w, "idxb": idxb_w})
        # host-side dequant constants (same corner-bound formula as device)
        c0, ca, cb, cab = _coef_from_weights(wsl)
        corners = np.stack([c0, c0 + ca, c0 + cb, c0 + ca + cb + cab], 0)
        mn = corners.min(0)
        rng = corners.max(0) - mn
        step = (rng + EPS) / OQ_RANGE
        off = mn + OQ_HALF * step
        deq.append((step.astype(np.float32), off.astype(np.float32)))
    return in_maps, (deq, perms)


def kernel(x, weights, idx_a, idx_b):
    x = np.asarray(x)
    out_dtype = x.dtype
    if "nc" not in _CACHE:
        _CACHE["nc"] = _build_nc()
    nc = _CACHE["nc"]

    in_maps, (deq, perms) = _host_prep(x, weights, idx_a, idx_b)
    res = bass_utils.run_bass_kernel_spmd(nc, in_maps,
                                          core_ids=list(range(NCORES)))
    out = np.empty((B, OUT_DIM), dtype=out_dtype)
    nf = (NT - N_I8) * 128
    for c in range(NCORES):
        lo = c * OUTC
        r = res.results[c]
        cols = np.empty((OUTC, B), dtype=np.float32)
        if nf:
            cols[:nf] = r["outT"]
        if nf < OUTC:
            step, off = deq[c]
            oq = r["outT8"].astype(np.float32)
            cols[nf:] = oq * step[nf:, None] + off[nf:, None]
        out[:, lo + perms[c]] = cols.T
    return out
